# revision 64
# baseline (speedup 1.0000x reference)
"""Trainium2 Bass kernel for ActivationRealQuantLinear.

Math (reference):
  per-token asymmetric 8-bit activation quant:
    xs = clip((max-min)/255, 1e-5), zp = clip(round(-min/xs), 0, 255)
    q  = clip(round(x/xs) + zp, 0, 255)
  grouped uint4 weight dequant: wdq[o,k] = (qw[o,k] - wzp[o,g]) * wsc[o,g]
  out[s,o] = (q @ wdq.T - zp[s]*wsum[o]) * xs[s] + bias[o]

Distribution (8 NeuronCores, one TRN2 chip):
  - out_features tensor-parallel: each core owns a 512-wide o-slice.
  - activation quant is token-sharded: each core quantizes its own 256
    tokens as two 128-token halves; each half's uint8 codes AllGather
    across all 8 cores as soon as they are staged, overlapping the
    weight dequant/transpose phase and the local-tile matmuls. Quant
    metadata (xs hi/lo + zp as bf16) rides in slot KC of the gathered
    buffer, so exactly two collectives are used (each costs ~40us in
    mesh handshakes + transfer regardless of payload size).
  - own tiles matmul straight from the SBUF cxT tiles; remote tiles
    load gathered codes with (pid + j) % 8 rotated addressing so the
    graph stays SPMD-identical (NOTE: compound pid expressions with
    subtraction, e.g. pid - pid%4 + ..., hang the HW runtime).
  - weights are dequantized on DVE/ACT (fused qw*s + (-zp*s)) and
    transposed on the PE early (PE pipeline fill); matmul in bf16 with
    fp32 PSUM accumulation; the zero-point correction is applied as a
    rank-1 update after the matmul; x loads are dispatched at queue
    heads (sync/scalar split) and weight loads ride the gpsimd queue.
  - NOTE: DMA transposes must stay on the sync queue — the scalar-queue
    xbar transpose silently corrupts data on HW — and an in-flight
    collective blocks HWDGE transposes, so both halves' transposes are
    scheduled before the collectives' data movement begins.
"""

import os
import sys

if "/opt/trn_rl_repo" not in sys.path:
    sys.path.insert(0, "/opt/trn_rl_repo")

import numpy as np
import ml_dtypes

import concourse.bacc as bacc
import concourse.bass as bass
import concourse.mybir as mybir
import concourse.tile as tile
import concourse.masks as masks
from concourse.bass_utils import run_bass_kernel_spmd

NCORES = 8
S, K, O = 2048, 4096, 4096
SL = S // NCORES          # 256 tokens owned per core
NLOC = 1                  # shares quantized locally (own only)
NFOR = NLOC - 1
OL = O // NCORES          # 512 out features per core
G = 32                    # weight quant groups
KC = K // 128             # 32 k-chunks of 128
MAGIC = float(1.5 * 2 ** 23)   # fp32 round-to-nearest-even trick
F32 = mybir.dt.float32
BF16 = mybir.dt.bfloat16
U8 = mybir.dt.uint8

_GRAPH = None
LAST_RESULTS = None


def _build():
    nc = bacc.Bacc("TRN2", target_bir_lowering=False, debug=False,
                   num_devices=NCORES)

    x_p = nc.declare_dram_parameter("x_loc", [NLOC * SL, K], F32,
                                    isOutput=False)
    qw_p = nc.declare_dram_parameter("qw", [OL, K], BF16, isOutput=False)
    wsc_p = nc.declare_dram_parameter("wsc", [OL, G], F32, isOutput=False)
    wzp_p = nc.declare_dram_parameter("wzp", [OL, G], F32, isOutput=False)
    b_p = nc.declare_dram_parameter("bias", [1, OL], F32, isOutput=False)
    out_p = nc.declare_dram_parameter("out", [S, OL], F32, isOutput=True)

    # own-share staging (whole tensors: they feed the collectives);
    # slot KC bytes 0..5 = metadata (xs_hi, xs_lo, zp as bf16)
    cxt_own = [nc.dram_tensor(f"cxt_own{h}", [128, KC + 1, 128], U8)
               for h in range(2)]
    cxt_all = [nc.dram_tensor(f"cxt_all{h}", [NCORES, 128, KC + 1, 128],
                              U8, addr_space="Shared") for h in range(2)]
    # foreign shares (locally quantized, no collective, no meta slot)
    cxt_for = (nc.dram_tensor("cxt_for", [NFOR, 2, 128, KC, 128], U8)
               if NFOR else None)

    groups_all = [list(range(NCORES))]
    Alu = mybir.AluOpType

    with tile.TileContext(nc) as tc:
        with (
            tc.tile_pool(name="persist", bufs=1) as persist,
            tc.tile_pool(name="xin", bufs=2) as xinp,
            tc.tile_pool(name="cxp", bufs=2) as cxp,
            tc.tile_pool(name="cxtk", bufs=2) as cxtk,
            tc.tile_pool(name="wtile", bufs=2) as wpool,
            tc.tile_pool(name="wdqp", bufs=3) as wdqp,
            tc.tile_pool(name="small", bufs=6) as small,
            tc.tile_pool(name="qmeta", bufs=6) as qmeta,
            tc.tile_pool(name="wsmall", bufs=12) as wsmall,
            tc.tile_pool(name="mm", bufs=3) as mmp,
            tc.tile_pool(name="out", bufs=2) as opool,
            tc.tile_pool(name="psum", bufs=4, space="PSUM") as psp,
        ):
            # ------- persistent tiles -------
            wdqT = persist.tile([128, KC, OL], BF16)        # 4 MB resident
            ones_col = persist.tile([1, 128], F32)
            nc.vector.memset(ones_col[:], 1.0)
            bias_bcast = persist.tile([128, OL], F32)
            magic_col = persist.tile([128, 1], F32)
            nc.vector.memset(magic_col[:], MAGIC)
            ident_bf = persist.tile([128, 128], BF16)
            masks.make_identity(nc, ident_bf[:])

            # ------- x loads dispatched first (queue heads) -------
            # own-share halves split sync/scalar; first foreign loads on
            # scalar; the last foreign pair is dispatched later (pool).
            x_ts = {}

            # x chunk boundaries: the first load is split across all
            # three DMA queues so the quant critical path starts ASAP;
            # partial reduces run per-chunk as the data lands.
            XC = [0, 1536, 3072, K]

            def load_x(slot, h, three_way=False):
                x_t = xinp.tile([128, K], F32, tag="xf32")
                r0 = slot * SL + h * 128
                if three_way:
                    nc.sync.dma_start(out=x_t[:, XC[0]:XC[1]],
                                      in_=x_p[r0:r0 + 128, XC[0]:XC[1]])
                    nc.scalar.dma_start(out=x_t[:, XC[1]:XC[2]],
                                        in_=x_p[r0:r0 + 128, XC[1]:XC[2]])
                    nc.gpsimd.dma_start(out=x_t[:, XC[2]:XC[3]],
                                        in_=x_p[r0:r0 + 128, XC[2]:XC[3]])
                elif slot == 0:
                    nc.sync.dma_start(out=x_t[:, 0:K // 2],
                                      in_=x_p[r0:r0 + 128, 0:K // 2])
                    nc.scalar.dma_start(out=x_t[:, K // 2:K],
                                        in_=x_p[r0:r0 + 128, K // 2:K])
                else:
                    nc.scalar.dma_start(out=x_t[:], in_=x_p[r0:r0 + 128, :])
                x_ts[(slot, h)] = x_t

            load_x(0, 0, three_way=True)
            load_x(0, 1)

            # ------- weight loads (gpsimd queue: scalar is x-loaded) ---
            qw_ts, wsc_ts, wzp_ts = [], [], []
            for oc in range(4):
                qw_t = wpool.tile([128, K], BF16, tag="qw")
                nc.gpsimd.dma_start(out=qw_t[:],
                                    in_=qw_p[oc * 128:(oc + 1) * 128, :])
                wsc_t = wsmall.tile([128, G], F32, tag="wsb")
                wzp_t = wsmall.tile([128, G], F32, tag="wsb")
                nc.gpsimd.dma_start(out=wsc_t[:],
                                    in_=wsc_p[oc * 128:(oc + 1) * 128, :])
                nc.gpsimd.dma_start(out=wzp_t[:],
                                    in_=wzp_p[oc * 128:(oc + 1) * 128, :])
                qw_ts.append(qw_t); wsc_ts.append(wsc_t); wzp_ts.append(wzp_t)

            wdq_ts = [None] * 4

            def dequant_oc(oc):
                qw_t, wsc_t, wzp_t = qw_ts[oc], wsc_ts[oc], wzp_ts[oc]
                nps = wsmall.tile([128, G], F32, tag="wsb")
                nc.vector.tensor_mul(nps[:], wzp_t[:], wsc_t[:])
                nc.vector.tensor_scalar(nps[:], nps[:], -1.0, None, Alu.mult)
                wdq = wdqp.tile([128, K], BF16, tag="wdq")
                for g in range(G):
                    sl = slice(g * 128, (g + 1) * 128)
                    if g % 2 == 0:
                        nc.vector.tensor_scalar(
                            wdq[:, sl], qw_t[:, sl], wsc_t[:, g:g + 1],
                            nps[:, g:g + 1], Alu.mult, Alu.add)
                    else:
                        nc.scalar.activation(
                            wdq[:, sl], qw_t[:, sl],
                            mybir.ActivationFunctionType.Identity,
                            bias=nps[:, g:g + 1], scale=wsc_t[:, g:g + 1])
                wdq_ts[oc] = wdq

            def transpose_oc(oc):
                wdq = wdq_ts[oc]
                for g in range(G):
                    sl = slice(g * 128, (g + 1) * 128)
                    ps_t = psp.tile([128, 128], BF16, tag="pst")
                    nc.tensor.matmul(ps_t[:], wdq[:, sl], ident_bf[:],
                                     is_transpose=True, start=True, stop=True)
                    if g % 2 == 0:
                        nc.vector.tensor_copy(
                            wdqT[:, g, oc * 128:(oc + 1) * 128], ps_t[:])
                    else:
                        nc.scalar.copy(
                            wdqT[:, g, oc * 128:(oc + 1) * 128], ps_t[:])

            # xs/zp/cxT tiles per (local slot, half), SBUF-resident
            xs_ts = [[None] * 2 for _ in range(NLOC)]
            zp_ts = [[None] * 2 for _ in range(NLOC)]
            cxT_ts = [[None] * 2 for _ in range(NLOC)]

            def quant_share(slot, h):
                """Quantize 128 tokens of local share `slot`, half `h`.
                slot 0 = own share: also stage metadata and the codes feed
                the collective; slots >=1: codes only, to cxt_for."""
                x_t = x_ts.pop((slot, h))
                xmin = small.tile([128, 1], F32, tag="st")
                xmax = small.tile([128, 1], F32, tag="st")
                if slot == 0 and h == 0:
                    # pipelined partial reduces per x chunk
                    pmin = small.tile([128, 3], F32, tag="pm")
                    pmax = small.tile([128, 3], F32, tag="pm")
                    for ci in range(3):
                        sl = slice(XC[ci], XC[ci + 1])
                        nc.vector.tensor_reduce(
                            pmin[:, ci:ci + 1], x_t[:, sl],
                            mybir.AxisListType.X, Alu.min)
                        nc.vector.tensor_reduce(
                            pmax[:, ci:ci + 1], x_t[:, sl],
                            mybir.AxisListType.X, Alu.max)
                    nc.vector.tensor_reduce(xmin[:], pmin[:],
                                            mybir.AxisListType.X, Alu.min)
                    nc.vector.tensor_reduce(xmax[:], pmax[:],
                                            mybir.AxisListType.X, Alu.max)
                else:
                    nc.vector.tensor_reduce(xmin[:], x_t[:],
                                            mybir.AxisListType.X, Alu.min)
                    nc.vector.tensor_reduce(xmax[:], x_t[:],
                                            mybir.AxisListType.X, Alu.max)
                xs = qmeta.tile([128, 1], F32, tag="xs")
                nc.vector.tensor_sub(xs[:], xmax[:], xmin[:])
                nc.vector.tensor_scalar(xs[:], xs[:], 1.0 / 255.0, 1e-5,
                                        Alu.mult, Alu.max)
                r = small.tile([128, 1], F32, tag="st")
                nc.vector.reciprocal(r[:], xs[:])
                t = small.tile([128, 1], F32, tag="st")
                nc.vector.tensor_mul(t[:], xs[:], r[:])
                nc.vector.tensor_scalar(t[:], t[:], 2.0, -1.0,
                                        Alu.subtract, Alu.mult)  # 2 - xs*r
                nc.vector.tensor_mul(r[:], r[:], t[:])
                zp = qmeta.tile([128, 1], F32, tag="zp")
                nc.vector.tensor_scalar(zp[:], xmin[:], -1.0, None, Alu.mult)
                nc.vector.tensor_mul(zp[:], zp[:], r[:])
                nc.vector.tensor_scalar(zp[:], zp[:], MAGIC, MAGIC,
                                        Alu.add, Alu.subtract)
                nc.vector.tensor_scalar(zp[:], zp[:], 0.0, 255.0,
                                        Alu.max, Alu.min)
                # q = clip(round(x*r) + zp, 0, 255)  (round via magic const)
                nc.scalar.activation(x_t[:], x_t[:],
                                     mybir.ActivationFunctionType.Identity,
                                     bias=magic_col[:], scale=r[:])
                nc.vector.tensor_scalar(x_t[:], x_t[:], MAGIC, zp[:],
                                        Alu.subtract, Alu.add)
                cx_sb = cxp.tile([128, K], BF16, tag="cx")
                nc.vector.tensor_scalar(cx_sb[:], x_t[:], 0.0, 255.0,
                                        Alu.max, Alu.min)
                # xbar transpose MUST be on sync (scalar xbar corrupts)
                cxT = cxtk.tile([128, KC, 128], BF16, tag="cxT")
                nc.sync.dma_start(out=cxT[:], in_=cx_sb[:], transpose=True)

                if slot == 0:
                    xs_hi_bf = small.tile([128, 1], BF16, tag="sb")
                    xs_hi_f = small.tile([128, 1], F32, tag="st")
                    nc.vector.tensor_copy(xs_hi_bf[:], xs[:])
                    nc.vector.tensor_copy(xs_hi_f[:], xs_hi_bf[:])
                    meta_bf = small.tile([128, 3], BF16, tag="meta")
                    nc.vector.tensor_copy(meta_bf[:, 0:1], xs_hi_bf[:])
                    nc.vector.tensor_sub(meta_bf[:, 1:2], xs[:], xs_hi_f[:])
                    nc.vector.tensor_copy(meta_bf[:, 2:3], zp[:])
                    nc.gpsimd.dma_start(out=cxt_own[h][:, 0:KC, :],
                                        in_=cxT[:])
                    nc.gpsimd.dma_start(out=cxt_own[h][:, KC, 0:6],
                                        in_=meta_bf[:].bitcast(U8))
                    nc.gpsimd.collective_compute(
                        "AllGather", Alu.bypass, replica_groups=groups_all,
                        ins=[cxt_own[h][:]], outs=[cxt_all[h][:]])
                else:
                    nc.gpsimd.dma_start(out=cxt_for[slot - 1, h],
                                        in_=cxT[:])
                xs_ts[slot][h], zp_ts[slot][h] = xs, zp
                cxT_ts[slot][h] = cxT

            # ================= emission schedule =================
            pid = nc.gpsimd.partition_id()
            # both quant halves first, before ANY dequant work: the
            # first AllGather's end time shifts the kernel end 1:1, and
            # half 1's transpose must beat the collective's blocking
            # window (in-flight collectives stall HWDGE transposes)
            quant_share(0, 0)        # own h0 -> stores + AllGather 0
            quant_share(0, 1)        # own h1 -> stores + AllGather 1
            dequant_oc(0)
            dequant_oc(1)
            transpose_oc(0)
            transpose_oc(1)
            dequant_oc(2)
            dequant_oc(3)
            transpose_oc(2)
            transpose_oc(3)

            # ------- wsum[o] broadcast rows via ones-matmul on PE ----
            ones_k = persist.tile([128, 128], BF16)
            nc.vector.memset(ones_k[:], 1.0)
            wsum_bcast = persist.tile([128, OL], F32)
            ps_w = psp.tile([128, OL], F32, tag="ps")
            for kc in range(KC):
                nc.tensor.matmul(ps_w[:], ones_k[:], wdqT[:, kc, :],
                                 start=(kc == 0), stop=(kc == KC - 1))
            nc.vector.tensor_copy(wsum_bcast[:], ps_w[:])

            # ------- bias broadcast (PE outer product) -------
            b_row = small.tile([1, OL], F32, tag="brow")
            nc.sync.dma_start(out=b_row[:], in_=b_p[:])
            ps_b = psp.tile([128, OL], F32, tag="ps")
            nc.tensor.matmul(ps_b[:], ones_col[:], b_row[:],
                             start=True, stop=True)
            nc.vector.tensor_copy(bias_bcast[:], ps_b[:])

            # ------- matmul tiles ----
            def mm_finish(ps, xs_t, zp_t, row0_ap):
                corr = opool.tile([128, OL], F32, tag="corr")
                nc.scalar.activation(corr[:], wsum_bcast[:],
                                     mybir.ActivationFunctionType.Identity,
                                     scale=zp_t[:])
                o_t = opool.tile([128, OL], F32, tag="ot")
                nc.vector.tensor_sub(o_t[:], ps[:], corr[:])
                nc.vector.tensor_scalar(o_t[:], o_t[:], xs_t[:],
                                        None, Alu.mult)
                nc.vector.tensor_add(o_t[:], o_t[:], bias_bcast[:])
                nc.gpsimd.dma_start(out=out_p[row0_ap, :], in_=o_t[:])

            def mm_local(slot, h):
                # matmul straight from the SBUF cxT tile (no roundtrip)
                lhsT = cxT_ts[slot][h]
                ps = psp.tile([128, OL], F32, tag="ps")
                for kc in range(KC):
                    nc.tensor.matmul(ps[:], lhsT[:, kc, :], wdqT[:, kc, :],
                                     start=(kc == 0), stop=(kc == KC - 1))
                if slot == 0:
                    row0 = pid * SL + h * 128
                else:
                    row0 = ((pid + slot) % NCORES) * SL + h * 128
                mm_finish(ps, xs_ts[slot][h], zp_ts[slot][h],
                          bass.ds(row0, 128))

            def mm_remote(j, h):
                share = (pid + j) % NCORES
                lhsT = mmp.tile([128, KC, 128], BF16, tag="lhsT")
                meta_u8 = small.tile([128, 6], U8, tag="mu8")
                nc.gpsimd.dma_start(
                    out=lhsT[:],
                    in_=cxt_all[h][bass.ds(share, 1), :, 0:KC, :])
                nc.gpsimd.dma_start(
                    out=meta_u8[:],
                    in_=cxt_all[h][bass.ds(share, 1), :, KC, 0:6])
                ps = psp.tile([128, OL], F32, tag="ps")
                for kc in range(KC):
                    nc.tensor.matmul(ps[:], lhsT[:, kc, :], wdqT[:, kc, :],
                                     start=(kc == 0), stop=(kc == KC - 1))
                meta = meta_u8[:].bitcast(BF16)
                xs_t = small.tile([128, 1], F32, tag="xst")
                nc.vector.tensor_add(xs_t[:], meta[:, 0:1], meta[:, 1:2])
                zp_t = small.tile([128, 1], F32, tag="xst")
                nc.vector.tensor_copy(zp_t[:], meta[:, 2:3])
                row0 = share * SL + h * 128
                mm_finish(ps, xs_t, zp_t, bass.ds(row0, 128))

            # own tiles, foreign tiles, then remote tiles (h0 before h1)
            for h in range(2):
                mm_local(0, h)
            for k in range(NFOR):
                for h in range(2):
                    mm_local(1 + k, h)
            for h in range(2):
                for j in range(NLOC, NCORES):
                    mm_remote(j, h)

    nc.compile()
    return nc


def _get_graph():
    global _GRAPH
    if _GRAPH is None:
        _GRAPH = _build()
    return _GRAPH


def kernel(x, qweight, w_scales, w_zero_points, bias):
    global LAST_RESULTS
    x2 = np.ascontiguousarray(np.asarray(x, np.float32).reshape(S, K))
    qw = np.ascontiguousarray(
        np.asarray(qweight).astype(ml_dtypes.bfloat16).reshape(O, K))
    wsc = np.ascontiguousarray(np.asarray(w_scales, np.float32))
    wzp = np.ascontiguousarray(np.asarray(w_zero_points).astype(np.float32))
    b = np.ascontiguousarray(np.asarray(bias, np.float32).reshape(1, O))

    in_maps = []
    for c in range(NCORES):
        shares = [(c + i) % NCORES for i in range(NLOC)]
        xloc = np.concatenate([x2[s * SL:(s + 1) * SL] for s in shares],
                              axis=0)
        in_maps.append({
            "x_loc": np.ascontiguousarray(xloc),
            "qw": np.ascontiguousarray(qw[c * OL:(c + 1) * OL]),
            "wsc": np.ascontiguousarray(wsc[c * OL:(c + 1) * OL]),
            "wzp": np.ascontiguousarray(wzp[c * OL:(c + 1) * OL]),
            "bias": np.ascontiguousarray(b[:, c * OL:(c + 1) * OL]),
        })

    nc = _get_graph()
    trace = os.environ.get("KTRACE", "0") == "1"
    res = run_bass_kernel_spmd(nc, in_maps, core_ids=list(range(NCORES)),
                               trace=trace)
    LAST_RESULTS = res
    outs = [np.asarray(res.results[c]["out"]) for c in range(NCORES)]
    return np.concatenate(outs, axis=1).reshape(1, S, O).astype(np.float32)


if __name__ == "__main__":
    rng = np.random.default_rng(0)
    x = rng.standard_normal((1, S, K), dtype=np.float32)
    qweight = rng.integers(0, 16, (O, G, 128), dtype=np.int32)
    w_scales = rng.uniform(0.001, 0.02, (O, G)).astype(np.float32)
    w_zero_points = rng.integers(0, 16, (O, G), dtype=np.int32)
    bias = rng.standard_normal(O).astype(np.float32)
    out = kernel(x=x, qweight=qweight, w_scales=w_scales,
                 w_zero_points=w_zero_points, bias=bias)
    print("out", out.shape, out.dtype, out[0, :2, :4])


# revision 67
# speedup vs baseline: 1.0315x; 1.0315x over previous
"""Trainium2 Bass kernel for ActivationRealQuantLinear.

Math (reference):
  per-token asymmetric 8-bit activation quant:
    xs = clip((max-min)/255, 1e-5), zp = clip(round(-min/xs), 0, 255)
    q  = clip(round(x/xs) + zp, 0, 255)
  grouped uint4 weight dequant: wdq[o,k] = (qw[o,k] - wzp[o,g]) * wsc[o,g]
  out[s,o] = (q @ wdq.T - zp[s]*wsum[o]) * xs[s] + bias[o]

Distribution (8 NeuronCores, one TRN2 chip):
  - out_features tensor-parallel: each core owns a 512-wide o-slice.
  - activation quant is token-sharded: each core quantizes its own 256
    tokens as two 128-token halves; each half's uint8 codes AllGather
    across all 8 cores as soon as they are staged, overlapping the
    weight dequant/transpose phase and the local-tile matmuls. Quant
    metadata (xs hi/lo + zp as bf16) rides in slot KC of the gathered
    buffer, so exactly two collectives are used (each costs ~40us in
    mesh handshakes + transfer regardless of payload size).
  - own tiles matmul straight from the SBUF cxT tiles; remote tiles
    load gathered codes with (pid + j) % 8 rotated addressing so the
    graph stays SPMD-identical (NOTE: compound pid expressions with
    subtraction, e.g. pid - pid%4 + ..., hang the HW runtime).
  - weights are dequantized on DVE/ACT (fused qw*s + (-zp*s)) and
    transposed on the PE early (PE pipeline fill); matmul in bf16 with
    fp32 PSUM accumulation; the zero-point correction is applied as a
    rank-1 update after the matmul; x loads are dispatched at queue
    heads (sync/scalar split) and weight loads ride the gpsimd queue.
  - NOTE: DMA transposes must stay on the sync queue — the scalar-queue
    xbar transpose silently corrupts data on HW — and an in-flight
    collective blocks HWDGE transposes, so both halves' transposes are
    scheduled before the collectives' data movement begins.
"""

import os
import sys

if "/opt/trn_rl_repo" not in sys.path:
    sys.path.insert(0, "/opt/trn_rl_repo")

import numpy as np
import ml_dtypes

import concourse.bacc as bacc
import concourse.bass as bass
import concourse.mybir as mybir
import concourse.tile as tile
import concourse.masks as masks
from concourse.bass_utils import run_bass_kernel_spmd

NCORES = 8
S, K, O = 2048, 4096, 4096
SL = S // NCORES          # 256 tokens owned per core
NLOC = 1                  # shares quantized locally (own only)
NFOR = NLOC - 1
OL = O // NCORES          # 512 out features per core
G = 32                    # weight quant groups
KC = K // 128             # 32 k-chunks of 128
MAGIC = float(1.5 * 2 ** 23)   # fp32 round-to-nearest-even trick
F32 = mybir.dt.float32
BF16 = mybir.dt.bfloat16
U8 = mybir.dt.uint8

_GRAPH = None
LAST_RESULTS = None


def _build():
    nc = bacc.Bacc("TRN2", target_bir_lowering=False, debug=False,
                   num_devices=NCORES)

    x_p = nc.declare_dram_parameter("x_loc", [NLOC * SL, K], F32,
                                    isOutput=False)
    qw_p = nc.declare_dram_parameter("qw", [OL, K], BF16, isOutput=False)
    wsc_p = nc.declare_dram_parameter("wsc", [OL, G], F32, isOutput=False)
    wzp_p = nc.declare_dram_parameter("wzp", [OL, G], F32, isOutput=False)
    b_p = nc.declare_dram_parameter("bias", [1, OL], F32, isOutput=False)
    out_p = nc.declare_dram_parameter("out", [S, OL], F32, isOutput=True)

    # own-share staging (whole tensors: they feed the collectives);
    # slot KC bytes 0..5 = metadata (xs_hi, xs_lo, zp as bf16)
    cxt_own = [nc.dram_tensor(f"cxt_own{h}", [128, KC + 1, 128], U8)
               for h in range(2)]
    cxt_all = [nc.dram_tensor(f"cxt_all{h}", [NCORES, 128, KC + 1, 128],
                              U8, addr_space="Shared") for h in range(2)]
    # foreign shares (locally quantized, no collective, no meta slot)
    cxt_for = (nc.dram_tensor("cxt_for", [NFOR, 2, 128, KC, 128], U8)
               if NFOR else None)

    groups_all = [list(range(NCORES))]
    Alu = mybir.AluOpType

    with tile.TileContext(nc) as tc:
        with (
            tc.tile_pool(name="persist", bufs=1) as persist,
            tc.tile_pool(name="xin", bufs=2) as xinp,
            tc.tile_pool(name="cxp", bufs=2) as cxp,
            tc.tile_pool(name="cxtk", bufs=2) as cxtk,
            tc.tile_pool(name="wtile", bufs=2) as wpool,
            tc.tile_pool(name="wdqp", bufs=3) as wdqp,
            tc.tile_pool(name="small", bufs=6) as small,
            tc.tile_pool(name="qmeta", bufs=6) as qmeta,
            tc.tile_pool(name="wsmall", bufs=12) as wsmall,
            tc.tile_pool(name="mm", bufs=3) as mmp,
            tc.tile_pool(name="out", bufs=2) as opool,
            tc.tile_pool(name="psum", bufs=4, space="PSUM") as psp,
        ):
            # ------- persistent tiles -------
            wdqT = persist.tile([128, KC, OL], BF16)        # 4 MB resident
            ones_col = persist.tile([1, 128], F32)
            nc.vector.memset(ones_col[:], 1.0)
            bias_bcast = persist.tile([128, OL], F32)
            magic_col = persist.tile([128, 1], F32)
            nc.vector.memset(magic_col[:], MAGIC)
            ident_bf = persist.tile([128, 128], BF16)
            masks.make_identity(nc, ident_bf[:])

            # ------- x loads dispatched first (queue heads) -------
            # own-share halves split sync/scalar; first foreign loads on
            # scalar; the last foreign pair is dispatched later (pool).
            x_ts = {}

            # x chunk boundaries: the first load is split across all
            # three DMA queues so the quant critical path starts ASAP;
            # partial reduces run per-chunk as the data lands.
            XC = [0, 1536, 3072, K]

            def load_x(slot, h, three_way=False):
                x_t = xinp.tile([128, K], F32, tag="xf32")
                r0 = slot * SL + h * 128
                if three_way:
                    nc.sync.dma_start(out=x_t[:, XC[0]:XC[1]],
                                      in_=x_p[r0:r0 + 128, XC[0]:XC[1]])
                    nc.scalar.dma_start(out=x_t[:, XC[1]:XC[2]],
                                        in_=x_p[r0:r0 + 128, XC[1]:XC[2]])
                    nc.gpsimd.dma_start(out=x_t[:, XC[2]:XC[3]],
                                        in_=x_p[r0:r0 + 128, XC[2]:XC[3]])
                elif slot == 0:
                    nc.sync.dma_start(out=x_t[:, 0:K // 2],
                                      in_=x_p[r0:r0 + 128, 0:K // 2])
                    nc.scalar.dma_start(out=x_t[:, K // 2:K],
                                        in_=x_p[r0:r0 + 128, K // 2:K])
                else:
                    nc.scalar.dma_start(out=x_t[:], in_=x_p[r0:r0 + 128, :])
                x_ts[(slot, h)] = x_t

            load_x(0, 0, three_way=True)
            load_x(0, 1)

            # ------- weight loads (gpsimd queue: scalar is x-loaded) ---
            qw_ts, wsc_ts, wzp_ts = [], [], []
            for oc in range(4):
                qw_t = wpool.tile([128, K], BF16, tag="qw")
                nc.gpsimd.dma_start(out=qw_t[:],
                                    in_=qw_p[oc * 128:(oc + 1) * 128, :])
                wsc_t = wsmall.tile([128, G], F32, tag="wsb")
                wzp_t = wsmall.tile([128, G], F32, tag="wsb")
                nc.gpsimd.dma_start(out=wsc_t[:],
                                    in_=wsc_p[oc * 128:(oc + 1) * 128, :])
                nc.gpsimd.dma_start(out=wzp_t[:],
                                    in_=wzp_p[oc * 128:(oc + 1) * 128, :])
                qw_ts.append(qw_t); wsc_ts.append(wsc_t); wzp_ts.append(wzp_t)

            wdq_ts = [None] * 4

            def dequant_oc(oc):
                qw_t, wsc_t, wzp_t = qw_ts[oc], wsc_ts[oc], wzp_ts[oc]
                nps = wsmall.tile([128, G], F32, tag="wsb")
                nc.vector.tensor_mul(nps[:], wzp_t[:], wsc_t[:])
                nc.vector.tensor_scalar(nps[:], nps[:], -1.0, None, Alu.mult)
                wdq = wdqp.tile([128, K], BF16, tag="wdq")
                for g in range(G):
                    sl = slice(g * 128, (g + 1) * 128)
                    if g % 2 == 0:
                        nc.vector.tensor_scalar(
                            wdq[:, sl], qw_t[:, sl], wsc_t[:, g:g + 1],
                            nps[:, g:g + 1], Alu.mult, Alu.add)
                    else:
                        nc.scalar.activation(
                            wdq[:, sl], qw_t[:, sl],
                            mybir.ActivationFunctionType.Identity,
                            bias=nps[:, g:g + 1], scale=wsc_t[:, g:g + 1])
                wdq_ts[oc] = wdq

            def transpose_oc(oc):
                wdq = wdq_ts[oc]
                for g in range(G):
                    sl = slice(g * 128, (g + 1) * 128)
                    ps_t = psp.tile([128, 128], BF16, tag="pst")
                    nc.tensor.matmul(ps_t[:], wdq[:, sl], ident_bf[:],
                                     is_transpose=True, start=True, stop=True)
                    if g % 2 == 0:
                        nc.vector.tensor_copy(
                            wdqT[:, g, oc * 128:(oc + 1) * 128], ps_t[:])
                    else:
                        nc.scalar.copy(
                            wdqT[:, g, oc * 128:(oc + 1) * 128], ps_t[:])

            # xs/zp/cxT tiles per (local slot, half), SBUF-resident
            xs_ts = [[None] * 2 for _ in range(NLOC)]
            zp_ts = [[None] * 2 for _ in range(NLOC)]
            cxT_ts = [[None] * 2 for _ in range(NLOC)]

            def quant_share(slot, h):
                """Quantize 128 tokens of local share `slot`, half `h`.
                slot 0 = own share: also stage metadata and the codes feed
                the collective; slots >=1: codes only, to cxt_for."""
                x_t = x_ts.pop((slot, h))
                xmin = small.tile([128, 1], F32, tag="st")
                xmax = small.tile([128, 1], F32, tag="st")
                if slot == 0 and h == 0:
                    # pipelined partial reduces per x chunk
                    pmin = small.tile([128, 3], F32, tag="pm")
                    pmax = small.tile([128, 3], F32, tag="pm")
                    for ci in range(3):
                        sl = slice(XC[ci], XC[ci + 1])
                        nc.vector.tensor_reduce(
                            pmin[:, ci:ci + 1], x_t[:, sl],
                            mybir.AxisListType.X, Alu.min)
                        nc.vector.tensor_reduce(
                            pmax[:, ci:ci + 1], x_t[:, sl],
                            mybir.AxisListType.X, Alu.max)
                    nc.vector.tensor_reduce(xmin[:], pmin[:],
                                            mybir.AxisListType.X, Alu.min)
                    nc.vector.tensor_reduce(xmax[:], pmax[:],
                                            mybir.AxisListType.X, Alu.max)
                else:
                    nc.vector.tensor_reduce(xmin[:], x_t[:],
                                            mybir.AxisListType.X, Alu.min)
                    nc.vector.tensor_reduce(xmax[:], x_t[:],
                                            mybir.AxisListType.X, Alu.max)
                xs = qmeta.tile([128, 1], F32, tag="xs")
                nc.vector.tensor_sub(xs[:], xmax[:], xmin[:])
                nc.vector.tensor_scalar(xs[:], xs[:], 1.0 / 255.0, 1e-5,
                                        Alu.mult, Alu.max)
                r = small.tile([128, 1], F32, tag="st")
                nc.vector.reciprocal(r[:], xs[:])
                t = small.tile([128, 1], F32, tag="st")
                nc.vector.tensor_mul(t[:], xs[:], r[:])
                nc.vector.tensor_scalar(t[:], t[:], 2.0, -1.0,
                                        Alu.subtract, Alu.mult)  # 2 - xs*r
                nc.vector.tensor_mul(r[:], r[:], t[:])
                zp = qmeta.tile([128, 1], F32, tag="zp")
                nc.vector.tensor_scalar(zp[:], xmin[:], -1.0, None, Alu.mult)
                nc.vector.tensor_mul(zp[:], zp[:], r[:])
                nc.vector.tensor_scalar(zp[:], zp[:], MAGIC, MAGIC,
                                        Alu.add, Alu.subtract)
                nc.vector.tensor_scalar(zp[:], zp[:], 0.0, 255.0,
                                        Alu.max, Alu.min)
                # q = clip(round(x*r) + zp, 0, 255)  (round via magic const)
                nc.scalar.activation(x_t[:], x_t[:],
                                     mybir.ActivationFunctionType.Identity,
                                     bias=magic_col[:], scale=r[:])
                nc.vector.tensor_scalar(x_t[:], x_t[:], MAGIC, zp[:],
                                        Alu.subtract, Alu.add)
                cx_sb = cxp.tile([128, K], BF16, tag="cx")
                nc.vector.tensor_scalar(cx_sb[:], x_t[:], 0.0, 255.0,
                                        Alu.max, Alu.min)
                # xbar transpose MUST be on sync (scalar xbar corrupts)
                cxT = cxtk.tile([128, KC, 128], BF16, tag="cxT")
                nc.sync.dma_start(out=cxT[:], in_=cx_sb[:], transpose=True)

                if slot == 0:
                    xs_hi_bf = small.tile([128, 1], BF16, tag="sb")
                    xs_hi_f = small.tile([128, 1], F32, tag="st")
                    nc.vector.tensor_copy(xs_hi_bf[:], xs[:])
                    nc.vector.tensor_copy(xs_hi_f[:], xs_hi_bf[:])
                    meta_bf = small.tile([128, 3], BF16, tag="meta")
                    nc.vector.tensor_copy(meta_bf[:, 0:1], xs_hi_bf[:])
                    nc.vector.tensor_sub(meta_bf[:, 1:2], xs[:], xs_hi_f[:])
                    nc.vector.tensor_copy(meta_bf[:, 2:3], zp[:])
                    nc.gpsimd.dma_start(out=cxt_own[h][:, 0:KC, :],
                                        in_=cxT[:])
                    nc.gpsimd.dma_start(out=cxt_own[h][:, KC, 0:6],
                                        in_=meta_bf[:].bitcast(U8))
                    nc.gpsimd.collective_compute(
                        "AllGather", Alu.bypass, replica_groups=groups_all,
                        ins=[cxt_own[h][:]], outs=[cxt_all[h][:]])
                else:
                    nc.gpsimd.dma_start(out=cxt_for[slot - 1, h],
                                        in_=cxT[:])
                xs_ts[slot][h], zp_ts[slot][h] = xs, zp
                cxT_ts[slot][h] = cxT

            # ================= emission schedule =================
            pid = nc.gpsimd.partition_id()
            # both quant halves first, before ANY dequant work: the
            # first AllGather's end time shifts the kernel end 1:1, and
            # half 1's transpose must beat the collective's blocking
            # window (in-flight collectives stall HWDGE transposes)
            quant_share(0, 0)        # own h0 -> stores + AllGather 0
            quant_share(0, 1)        # own h1 -> stores + AllGather 1
            dequant_oc(0)
            dequant_oc(1)
            transpose_oc(0)
            transpose_oc(1)
            dequant_oc(2)
            dequant_oc(3)
            transpose_oc(2)
            transpose_oc(3)

            # ------- wsum[o] broadcast rows via ones-matmul on PE ----
            ones_k = persist.tile([128, 128], BF16)
            nc.vector.memset(ones_k[:], 1.0)
            wsum_bcast = persist.tile([128, OL], F32)
            ps_w = psp.tile([128, OL], F32, tag="ps")
            for kc in range(KC):
                nc.tensor.matmul(ps_w[:], ones_k[:], wdqT[:, kc, :],
                                 start=(kc == 0), stop=(kc == KC - 1))
            nc.vector.tensor_copy(wsum_bcast[:], ps_w[:])

            # ------- bias broadcast (PE outer product) -------
            b_row = small.tile([1, OL], F32, tag="brow")
            nc.sync.dma_start(out=b_row[:], in_=b_p[:])
            ps_b = psp.tile([128, OL], F32, tag="ps")
            nc.tensor.matmul(ps_b[:], ones_col[:], b_row[:],
                             start=True, stop=True)
            nc.vector.tensor_copy(bias_bcast[:], ps_b[:])

            # ------- matmul tiles ----
            def mm_finish(ps, xs_t, zp_t, row0_ap):
                corr = opool.tile([128, OL], F32, tag="corr")
                nc.scalar.activation(corr[:], wsum_bcast[:],
                                     mybir.ActivationFunctionType.Identity,
                                     scale=zp_t[:])
                o_t = opool.tile([128, OL], F32, tag="ot")
                nc.vector.tensor_sub(o_t[:], ps[:], corr[:])
                nc.vector.tensor_scalar(o_t[:], o_t[:], xs_t[:],
                                        None, Alu.mult)
                nc.vector.tensor_add(o_t[:], o_t[:], bias_bcast[:])
                nc.gpsimd.dma_start(out=out_p[row0_ap, :], in_=o_t[:])

            def mm_local(slot, h):
                # matmul straight from the SBUF cxT tile (no roundtrip)
                lhsT = cxT_ts[slot][h]
                ps = psp.tile([128, OL], F32, tag="ps")
                for kc in range(KC):
                    nc.tensor.matmul(ps[:], lhsT[:, kc, :], wdqT[:, kc, :],
                                     start=(kc == 0), stop=(kc == KC - 1))
                if slot == 0:
                    row0 = pid * SL + h * 128
                else:
                    row0 = ((pid + slot) % NCORES) * SL + h * 128
                mm_finish(ps, xs_ts[slot][h], zp_ts[slot][h],
                          bass.ds(row0, 128))

            def mm_remote(j, h):
                share = (pid + j) % NCORES
                lhsT = mmp.tile([128, KC, 128], BF16, tag="lhsT")
                meta_u8 = small.tile([128, 6], U8, tag="mu8")
                nc.gpsimd.dma_start(
                    out=lhsT[:],
                    in_=cxt_all[h][bass.ds(share, 1), :, 0:KC, :])
                nc.gpsimd.dma_start(
                    out=meta_u8[:],
                    in_=cxt_all[h][bass.ds(share, 1), :, KC, 0:6])
                ps = psp.tile([128, OL], F32, tag="ps")
                for kc in range(KC):
                    nc.tensor.matmul(ps[:], lhsT[:, kc, :], wdqT[:, kc, :],
                                     start=(kc == 0), stop=(kc == KC - 1))
                meta = meta_u8[:].bitcast(BF16)
                xs_t = small.tile([128, 1], F32, tag="xst")
                nc.vector.tensor_add(xs_t[:], meta[:, 0:1], meta[:, 1:2])
                zp_t = small.tile([128, 1], F32, tag="xst")
                nc.vector.tensor_copy(zp_t[:], meta[:, 2:3])
                row0 = share * SL + h * 128
                mm_finish(ps, xs_t, zp_t, bass.ds(row0, 128))

            # own tiles, foreign tiles, then remote tiles (h0 before h1)
            for h in range(2):
                mm_local(0, h)
            for k in range(NFOR):
                for h in range(2):
                    mm_local(1 + k, h)
            for h in range(2):
                for j in range(NLOC, NCORES):
                    mm_remote(j, h)

    nc.compile()
    return nc


def _get_graph():
    global _GRAPH
    if _GRAPH is None:
        _GRAPH = _build()
    return _GRAPH


def kernel(x, qweight, w_scales, w_zero_points, bias):
    global LAST_RESULTS
    x2 = np.ascontiguousarray(np.asarray(x, np.float32).reshape(S, K))
    qw = np.ascontiguousarray(
        np.asarray(qweight).astype(ml_dtypes.bfloat16).reshape(O, K))
    wsc = np.ascontiguousarray(np.asarray(w_scales, np.float32))
    wzp = np.ascontiguousarray(np.asarray(w_zero_points).astype(np.float32))
    b = np.ascontiguousarray(np.asarray(bias, np.float32).reshape(1, O))

    in_maps = []
    for c in range(NCORES):
        shares = [(c + i) % NCORES for i in range(NLOC)]
        xloc = np.concatenate([x2[s * SL:(s + 1) * SL] for s in shares],
                              axis=0)
        in_maps.append({
            "x_loc": np.ascontiguousarray(xloc),
            "qw": np.ascontiguousarray(qw[c * OL:(c + 1) * OL]),
            "wsc": np.ascontiguousarray(wsc[c * OL:(c + 1) * OL]),
            "wzp": np.ascontiguousarray(wzp[c * OL:(c + 1) * OL]),
            "bias": np.ascontiguousarray(b[:, c * OL:(c + 1) * OL]),
        })

    nc = _get_graph()
    trace = os.environ.get("KTRACE", "0") == "1"
    res = run_bass_kernel_spmd(nc, in_maps, core_ids=list(range(NCORES)),
                               trace=trace)
    LAST_RESULTS = res
    outs = [np.asarray(res.results[c]["out"]) for c in range(NCORES)]
    return np.concatenate(outs, axis=1).reshape(1, S, O).astype(np.float32)


if __name__ == "__main__":
    rng = np.random.default_rng(0)
    x = rng.standard_normal((1, S, K), dtype=np.float32)
    qweight = rng.integers(0, 16, (O, G, 128), dtype=np.int32)
    w_scales = rng.uniform(0.001, 0.02, (O, G)).astype(np.float32)
    w_zero_points = rng.integers(0, 16, (O, G), dtype=np.int32)
    bias = rng.standard_normal(O).astype(np.float32)
    out = kernel(x=x, qweight=qweight, w_scales=w_scales,
                 w_zero_points=w_zero_points, bias=bias)
    print("out", out.shape, out.dtype, out[0, :2, :4])


# revision 68
# speedup vs baseline: 1.0468x; 1.0149x over previous
"""Trainium2 Bass kernel for ActivationRealQuantLinear.

Math (reference):
  per-token asymmetric 8-bit activation quant:
    xs = clip((max-min)/255, 1e-5), zp = clip(round(-min/xs), 0, 255)
    q  = clip(round(x/xs) + zp, 0, 255)
  grouped uint4 weight dequant: wdq[o,k] = (qw[o,k] - wzp[o,g]) * wsc[o,g]
  out[s,o] = (q @ wdq.T - zp[s]*wsum[o]) * xs[s] + bias[o]

Distribution (8 NeuronCores, one TRN2 chip):
  - out_features tensor-parallel: each core owns a 512-wide o-slice.
  - activation quant is token-sharded: each core quantizes its own 256
    tokens as two 128-token halves; each half's uint8 codes AllGather
    across all 8 cores as soon as they are staged, overlapping the
    weight dequant/transpose phase and the local-tile matmuls. Quant
    metadata (xs hi/lo + zp as bf16) rides in slot KC of the gathered
    buffer, so exactly two collectives are used (each costs ~40us in
    mesh handshakes + transfer regardless of payload size).
  - own tiles matmul straight from the SBUF cxT tiles; remote tiles
    load gathered codes with (pid + j) % 8 rotated addressing so the
    graph stays SPMD-identical (NOTE: compound pid expressions with
    subtraction, e.g. pid - pid%4 + ..., hang the HW runtime).
  - weights are dequantized on DVE/ACT (fused qw*s + (-zp*s)) and
    transposed on the PE early (PE pipeline fill); matmul in bf16 with
    fp32 PSUM accumulation; the zero-point correction is applied as a
    rank-1 update after the matmul; x loads are dispatched at queue
    heads (sync/scalar split) and weight loads ride the gpsimd queue.
  - NOTE: DMA transposes must stay on the sync queue — the scalar-queue
    xbar transpose silently corrupts data on HW — and an in-flight
    collective blocks HWDGE transposes, so both halves' transposes are
    scheduled before the collectives' data movement begins.
"""

import os
import sys

if "/opt/trn_rl_repo" not in sys.path:
    sys.path.insert(0, "/opt/trn_rl_repo")

import numpy as np
import ml_dtypes

import concourse.bacc as bacc
import concourse.bass as bass
import concourse.mybir as mybir
import concourse.tile as tile
import concourse.masks as masks
from concourse.bass_utils import run_bass_kernel_spmd

NCORES = 8
S, K, O = 2048, 4096, 4096
SL = S // NCORES          # 256 tokens owned per core
NLOC = 1                  # shares quantized locally (own only)
NFOR = NLOC - 1
OL = O // NCORES          # 512 out features per core
G = 32                    # weight quant groups
KC = K // 128             # 32 k-chunks of 128
MAGIC = float(1.5 * 2 ** 23)   # fp32 round-to-nearest-even trick
F32 = mybir.dt.float32
BF16 = mybir.dt.bfloat16
U8 = mybir.dt.uint8

_GRAPH = None
LAST_RESULTS = None


def _build():
    nc = bacc.Bacc("TRN2", target_bir_lowering=False, debug=False,
                   num_devices=NCORES)

    x_p = nc.declare_dram_parameter("x_loc", [NLOC * SL, K], F32,
                                    isOutput=False)
    qw_p = nc.declare_dram_parameter("qw", [OL, K], BF16, isOutput=False)
    wsc_p = nc.declare_dram_parameter("wsc", [OL, G], F32, isOutput=False)
    wzp_p = nc.declare_dram_parameter("wzp", [OL, G], F32, isOutput=False)
    b_p = nc.declare_dram_parameter("bias", [1, OL], F32, isOutput=False)
    out_p = nc.declare_dram_parameter("out", [S, OL], F32, isOutput=True)

    # own-share staging (whole tensors: they feed the collectives);
    # slot KC bytes 0..5 = metadata (xs_hi, xs_lo, zp as bf16)
    cxt_own = [nc.dram_tensor(f"cxt_own{h}", [128, KC + 1, 128], U8)
               for h in range(2)]
    cxt_all = [nc.dram_tensor(f"cxt_all{h}", [NCORES, 128, KC + 1, 128],
                              U8, addr_space="Shared") for h in range(2)]
    # foreign shares (locally quantized, no collective, no meta slot)
    cxt_for = (nc.dram_tensor("cxt_for", [NFOR, 2, 128, KC, 128], U8)
               if NFOR else None)

    groups_all = [list(range(NCORES))]
    Alu = mybir.AluOpType

    with tile.TileContext(nc) as tc:
        with (
            tc.tile_pool(name="persist", bufs=1) as persist,
            tc.tile_pool(name="xin", bufs=2) as xinp,
            tc.tile_pool(name="cxp", bufs=2) as cxp,
            tc.tile_pool(name="cxtk", bufs=2) as cxtk,
            tc.tile_pool(name="wtile", bufs=2) as wpool,
            tc.tile_pool(name="wdqp", bufs=3) as wdqp,
            tc.tile_pool(name="small", bufs=6) as small,
            tc.tile_pool(name="qmeta", bufs=6) as qmeta,
            tc.tile_pool(name="wsmall", bufs=12) as wsmall,
            tc.tile_pool(name="mm", bufs=3) as mmp,
            tc.tile_pool(name="out", bufs=2) as opool,
            tc.tile_pool(name="psum", bufs=4, space="PSUM") as psp,
        ):
            # ------- persistent tiles -------
            wdqT = persist.tile([128, KC, OL], BF16)        # 4 MB resident
            ones_col = persist.tile([1, 128], F32)
            nc.vector.memset(ones_col[:], 1.0)
            bias_bcast = persist.tile([128, OL], F32)
            magic_col = persist.tile([128, 1], F32)
            nc.vector.memset(magic_col[:], MAGIC)
            ident_bf = persist.tile([128, 128], BF16)
            masks.make_identity(nc, ident_bf[:])

            # ------- x loads dispatched first (queue heads) -------
            # own-share halves split sync/scalar; first foreign loads on
            # scalar; the last foreign pair is dispatched later (pool).
            x_ts = {}

            # x chunk boundaries: the first load is split across all
            # three DMA queues so the quant critical path starts ASAP;
            # partial reduces run per-chunk as the data lands.
            XC = [0, 1536, 3072, K]

            def load_x(slot, h, three_way=False):
                x_t = xinp.tile([128, K], F32, tag="xf32")
                r0 = slot * SL + h * 128
                if three_way:
                    nc.sync.dma_start(out=x_t[:, XC[0]:XC[1]],
                                      in_=x_p[r0:r0 + 128, XC[0]:XC[1]])
                    nc.scalar.dma_start(out=x_t[:, XC[1]:XC[2]],
                                        in_=x_p[r0:r0 + 128, XC[1]:XC[2]])
                    nc.gpsimd.dma_start(out=x_t[:, XC[2]:XC[3]],
                                        in_=x_p[r0:r0 + 128, XC[2]:XC[3]])
                elif slot == 0:
                    nc.sync.dma_start(out=x_t[:, 0:K // 2],
                                      in_=x_p[r0:r0 + 128, 0:K // 2])
                    nc.scalar.dma_start(out=x_t[:, K // 2:K],
                                        in_=x_p[r0:r0 + 128, K // 2:K])
                else:
                    nc.scalar.dma_start(out=x_t[:], in_=x_p[r0:r0 + 128, :])
                x_ts[(slot, h)] = x_t

            load_x(0, 0, three_way=True)
            load_x(0, 1)

            # ------- weight loads (gpsimd queue: scalar is x-loaded) ---
            qw_ts, wsc_ts, wzp_ts = [], [], []
            for oc in range(4):
                qw_t = wpool.tile([128, K], BF16, tag="qw")
                nc.gpsimd.dma_start(out=qw_t[:],
                                    in_=qw_p[oc * 128:(oc + 1) * 128, :])
                wsc_t = wsmall.tile([128, G], F32, tag="wsb")
                wzp_t = wsmall.tile([128, G], F32, tag="wsb")
                nc.gpsimd.dma_start(out=wsc_t[:],
                                    in_=wsc_p[oc * 128:(oc + 1) * 128, :])
                nc.gpsimd.dma_start(out=wzp_t[:],
                                    in_=wzp_p[oc * 128:(oc + 1) * 128, :])
                qw_ts.append(qw_t); wsc_ts.append(wsc_t); wzp_ts.append(wzp_t)

            wdq_ts = [None] * 4

            def dequant_oc(oc):
                qw_t, wsc_t, wzp_t = qw_ts[oc], wsc_ts[oc], wzp_ts[oc]
                nps = wsmall.tile([128, G], F32, tag="wsb")
                nc.vector.tensor_mul(nps[:], wzp_t[:], wsc_t[:])
                nc.vector.tensor_scalar(nps[:], nps[:], -1.0, None, Alu.mult)
                wdq = wdqp.tile([128, K], BF16, tag="wdq")
                for g in range(G):
                    sl = slice(g * 128, (g + 1) * 128)
                    if g % 2 == 0:
                        nc.vector.tensor_scalar(
                            wdq[:, sl], qw_t[:, sl], wsc_t[:, g:g + 1],
                            nps[:, g:g + 1], Alu.mult, Alu.add)
                    else:
                        nc.scalar.activation(
                            wdq[:, sl], qw_t[:, sl],
                            mybir.ActivationFunctionType.Identity,
                            bias=nps[:, g:g + 1], scale=wsc_t[:, g:g + 1])
                wdq_ts[oc] = wdq

            def transpose_oc(oc):
                wdq = wdq_ts[oc]
                for g in range(G):
                    sl = slice(g * 128, (g + 1) * 128)
                    ps_t = psp.tile([128, 128], BF16, tag="pst")
                    nc.tensor.matmul(ps_t[:], wdq[:, sl], ident_bf[:],
                                     is_transpose=True, start=True, stop=True)
                    if g % 2 == 0:
                        nc.vector.tensor_copy(
                            wdqT[:, g, oc * 128:(oc + 1) * 128], ps_t[:])
                    else:
                        nc.scalar.copy(
                            wdqT[:, g, oc * 128:(oc + 1) * 128], ps_t[:])

            # xs/zp/cxT tiles per (local slot, half), SBUF-resident
            xs_ts = [[None] * 2 for _ in range(NLOC)]
            zp_ts = [[None] * 2 for _ in range(NLOC)]
            cxT_ts = [[None] * 2 for _ in range(NLOC)]

            def quant_share(slot, h):
                """Quantize 128 tokens of local share `slot`, half `h`.
                slot 0 = own share: also stage metadata and the codes feed
                the collective; slots >=1: codes only, to cxt_for."""
                x_t = x_ts.pop((slot, h))
                xmin = small.tile([128, 1], F32, tag="st")
                xmax = small.tile([128, 1], F32, tag="st")
                if slot == 0 and h == 0:
                    # pipelined partial reduces per x chunk
                    pmin = small.tile([128, 3], F32, tag="pm")
                    pmax = small.tile([128, 3], F32, tag="pm")
                    for ci in range(3):
                        sl = slice(XC[ci], XC[ci + 1])
                        nc.vector.tensor_reduce(
                            pmin[:, ci:ci + 1], x_t[:, sl],
                            mybir.AxisListType.X, Alu.min)
                        nc.vector.tensor_reduce(
                            pmax[:, ci:ci + 1], x_t[:, sl],
                            mybir.AxisListType.X, Alu.max)
                    nc.vector.tensor_reduce(xmin[:], pmin[:],
                                            mybir.AxisListType.X, Alu.min)
                    nc.vector.tensor_reduce(xmax[:], pmax[:],
                                            mybir.AxisListType.X, Alu.max)
                else:
                    nc.vector.tensor_reduce(xmin[:], x_t[:],
                                            mybir.AxisListType.X, Alu.min)
                    nc.vector.tensor_reduce(xmax[:], x_t[:],
                                            mybir.AxisListType.X, Alu.max)
                xs = qmeta.tile([128, 1], F32, tag="xs")
                nc.vector.tensor_sub(xs[:], xmax[:], xmin[:])
                nc.vector.tensor_scalar(xs[:], xs[:], 1.0 / 255.0, 1e-5,
                                        Alu.mult, Alu.max)
                r = small.tile([128, 1], F32, tag="st")
                nc.vector.reciprocal(r[:], xs[:])
                t = small.tile([128, 1], F32, tag="st")
                nc.vector.tensor_mul(t[:], xs[:], r[:])
                nc.vector.tensor_scalar(t[:], t[:], 2.0, -1.0,
                                        Alu.subtract, Alu.mult)  # 2 - xs*r
                nc.vector.tensor_mul(r[:], r[:], t[:])
                zp = qmeta.tile([128, 1], F32, tag="zp")
                nc.vector.tensor_scalar(zp[:], xmin[:], -1.0, None, Alu.mult)
                nc.vector.tensor_mul(zp[:], zp[:], r[:])
                nc.vector.tensor_scalar(zp[:], zp[:], MAGIC, MAGIC,
                                        Alu.add, Alu.subtract)
                nc.vector.tensor_scalar(zp[:], zp[:], 0.0, 255.0,
                                        Alu.max, Alu.min)
                # q = clip(round(x*r) + zp, 0, 255)  (round via magic const)
                if slot == 0:
                    # metadata first: off the critical quant->gather tail
                    xs_hi_bf = small.tile([128, 1], BF16, tag="sb")
                    xs_hi_f = small.tile([128, 1], F32, tag="st")
                    nc.vector.tensor_copy(xs_hi_bf[:], xs[:])
                    nc.vector.tensor_copy(xs_hi_f[:], xs_hi_bf[:])
                    meta_bf = small.tile([128, 3], BF16, tag="meta")
                    nc.vector.tensor_copy(meta_bf[:, 0:1], xs_hi_bf[:])
                    nc.vector.tensor_sub(meta_bf[:, 1:2], xs[:], xs_hi_f[:])
                    nc.vector.tensor_copy(meta_bf[:, 2:3], zp[:])
                    nc.gpsimd.dma_start(out=cxt_own[h][:, KC, 0:6],
                                        in_=meta_bf[:].bitcast(U8))

                cx_sb = cxp.tile([128, K], BF16, tag="cx")
                cxT = cxtk.tile([128, KC, 128], BF16, tag="cxT")
                # round/shift/clip + transpose + store pipelined per
                # k-half; xbar transposes MUST be on sync (scalar xbar
                # corrupts) but each half waits only its own clip
                for a, b in ((0, K // 2), (K // 2, K)):
                    ka, kb = a // 128, b // 128
                    nc.scalar.activation(x_t[:, a:b], x_t[:, a:b],
                                         mybir.ActivationFunctionType
                                         .Identity,
                                         bias=magic_col[:], scale=r[:])
                    nc.vector.tensor_scalar(x_t[:, a:b], x_t[:, a:b],
                                            MAGIC, zp[:],
                                            Alu.subtract, Alu.add)
                    nc.vector.tensor_scalar(cx_sb[:, a:b], x_t[:, a:b],
                                            0.0, 255.0, Alu.max, Alu.min)
                    nc.sync.dma_start(out=cxT[:, ka:kb, :],
                                      in_=cx_sb[:, a:b], transpose=True)
                    if slot == 0:
                        nc.gpsimd.dma_start(
                            out=cxt_own[h][:, ka:kb, :],
                            in_=cxT[:, ka:kb, :])
                    else:
                        nc.gpsimd.dma_start(
                            out=cxt_for[slot - 1, h][:, ka:kb, :],
                            in_=cxT[:, ka:kb, :])
                if slot == 0:
                    nc.gpsimd.collective_compute(
                        "AllGather", Alu.bypass, replica_groups=groups_all,
                        ins=[cxt_own[h][:]], outs=[cxt_all[h][:]])
                xs_ts[slot][h], zp_ts[slot][h] = xs, zp
                cxT_ts[slot][h] = cxT

            # ================= emission schedule =================
            pid = nc.gpsimd.partition_id()
            # both quant halves first, before ANY dequant work: the
            # first AllGather's end time shifts the kernel end 1:1, and
            # half 1's transpose must beat the collective's blocking
            # window (in-flight collectives stall HWDGE transposes)
            quant_share(0, 0)        # own h0 -> stores + AllGather 0
            quant_share(0, 1)        # own h1 -> stores + AllGather 1
            dequant_oc(0)
            dequant_oc(1)
            transpose_oc(0)
            transpose_oc(1)
            dequant_oc(2)
            dequant_oc(3)
            transpose_oc(2)
            transpose_oc(3)

            # ------- wsum[o] broadcast rows via ones-matmul on PE ----
            ones_k = persist.tile([128, 128], BF16)
            nc.vector.memset(ones_k[:], 1.0)
            wsum_bcast = persist.tile([128, OL], F32)
            ps_w = psp.tile([128, OL], F32, tag="ps")
            for kc in range(KC):
                nc.tensor.matmul(ps_w[:], ones_k[:], wdqT[:, kc, :],
                                 start=(kc == 0), stop=(kc == KC - 1))
            nc.vector.tensor_copy(wsum_bcast[:], ps_w[:])

            # ------- bias broadcast (PE outer product) -------
            b_row = small.tile([1, OL], F32, tag="brow")
            nc.sync.dma_start(out=b_row[:], in_=b_p[:])
            ps_b = psp.tile([128, OL], F32, tag="ps")
            nc.tensor.matmul(ps_b[:], ones_col[:], b_row[:],
                             start=True, stop=True)
            nc.vector.tensor_copy(bias_bcast[:], ps_b[:])

            # ------- matmul tiles ----
            def mm_finish(ps, xs_t, zp_t, row0_ap):
                corr = opool.tile([128, OL], F32, tag="corr")
                nc.scalar.activation(corr[:], wsum_bcast[:],
                                     mybir.ActivationFunctionType.Identity,
                                     scale=zp_t[:])
                o_t = opool.tile([128, OL], F32, tag="ot")
                nc.vector.tensor_sub(o_t[:], ps[:], corr[:])
                nc.vector.tensor_scalar(o_t[:], o_t[:], xs_t[:],
                                        None, Alu.mult)
                nc.vector.tensor_add(o_t[:], o_t[:], bias_bcast[:])
                nc.gpsimd.dma_start(out=out_p[row0_ap, :], in_=o_t[:])

            def mm_local(slot, h):
                # matmul straight from the SBUF cxT tile (no roundtrip)
                lhsT = cxT_ts[slot][h]
                ps = psp.tile([128, OL], F32, tag="ps")
                for kc in range(KC):
                    nc.tensor.matmul(ps[:], lhsT[:, kc, :], wdqT[:, kc, :],
                                     start=(kc == 0), stop=(kc == KC - 1))
                if slot == 0:
                    row0 = pid * SL + h * 128
                else:
                    row0 = ((pid + slot) % NCORES) * SL + h * 128
                mm_finish(ps, xs_ts[slot][h], zp_ts[slot][h],
                          bass.ds(row0, 128))

            def mm_remote(j, h):
                share = (pid + j) % NCORES
                lhsT = mmp.tile([128, KC, 128], BF16, tag="lhsT")
                meta_u8 = small.tile([128, 6], U8, tag="mu8")
                nc.gpsimd.dma_start(
                    out=lhsT[:],
                    in_=cxt_all[h][bass.ds(share, 1), :, 0:KC, :])
                nc.gpsimd.dma_start(
                    out=meta_u8[:],
                    in_=cxt_all[h][bass.ds(share, 1), :, KC, 0:6])
                ps = psp.tile([128, OL], F32, tag="ps")
                for kc in range(KC):
                    nc.tensor.matmul(ps[:], lhsT[:, kc, :], wdqT[:, kc, :],
                                     start=(kc == 0), stop=(kc == KC - 1))
                meta = meta_u8[:].bitcast(BF16)
                xs_t = small.tile([128, 1], F32, tag="xst")
                nc.vector.tensor_add(xs_t[:], meta[:, 0:1], meta[:, 1:2])
                zp_t = small.tile([128, 1], F32, tag="xst")
                nc.vector.tensor_copy(zp_t[:], meta[:, 2:3])
                row0 = share * SL + h * 128
                mm_finish(ps, xs_t, zp_t, bass.ds(row0, 128))

            # own tiles, foreign tiles, then remote tiles (h0 before h1)
            for h in range(2):
                mm_local(0, h)
            for k in range(NFOR):
                for h in range(2):
                    mm_local(1 + k, h)
            for h in range(2):
                for j in range(NLOC, NCORES):
                    mm_remote(j, h)

    nc.compile()
    return nc


def _get_graph():
    global _GRAPH
    if _GRAPH is None:
        _GRAPH = _build()
    return _GRAPH


def kernel(x, qweight, w_scales, w_zero_points, bias):
    global LAST_RESULTS
    x2 = np.ascontiguousarray(np.asarray(x, np.float32).reshape(S, K))
    qw = np.ascontiguousarray(
        np.asarray(qweight).astype(ml_dtypes.bfloat16).reshape(O, K))
    wsc = np.ascontiguousarray(np.asarray(w_scales, np.float32))
    wzp = np.ascontiguousarray(np.asarray(w_zero_points).astype(np.float32))
    b = np.ascontiguousarray(np.asarray(bias, np.float32).reshape(1, O))

    in_maps = []
    for c in range(NCORES):
        shares = [(c + i) % NCORES for i in range(NLOC)]
        xloc = np.concatenate([x2[s * SL:(s + 1) * SL] for s in shares],
                              axis=0)
        in_maps.append({
            "x_loc": np.ascontiguousarray(xloc),
            "qw": np.ascontiguousarray(qw[c * OL:(c + 1) * OL]),
            "wsc": np.ascontiguousarray(wsc[c * OL:(c + 1) * OL]),
            "wzp": np.ascontiguousarray(wzp[c * OL:(c + 1) * OL]),
            "bias": np.ascontiguousarray(b[:, c * OL:(c + 1) * OL]),
        })

    nc = _get_graph()
    trace = os.environ.get("KTRACE", "0") == "1"
    res = run_bass_kernel_spmd(nc, in_maps, core_ids=list(range(NCORES)),
                               trace=trace)
    LAST_RESULTS = res
    outs = [np.asarray(res.results[c]["out"]) for c in range(NCORES)]
    return np.concatenate(outs, axis=1).reshape(1, S, O).astype(np.float32)


if __name__ == "__main__":
    rng = np.random.default_rng(0)
    x = rng.standard_normal((1, S, K), dtype=np.float32)
    qweight = rng.integers(0, 16, (O, G, 128), dtype=np.int32)
    w_scales = rng.uniform(0.001, 0.02, (O, G)).astype(np.float32)
    w_zero_points = rng.integers(0, 16, (O, G), dtype=np.int32)
    bias = rng.standard_normal(O).astype(np.float32)
    out = kernel(x=x, qweight=qweight, w_scales=w_scales,
                 w_zero_points=w_zero_points, bias=bias)
    print("out", out.shape, out.dtype, out[0, :2, :4])
